# revision 4
# baseline (speedup 1.0000x reference)
"""Trainium2 Bass kernel for space-to-depth (pixel-unshuffle, factor 2).

Input  x:   (8, 32, 512, 512) f32
Output out: (8, 128, 256, 256) f32 with out[b, 4i+2dh+dw, h, w] = x[b, i, 2h+dh, 2w+dw]

Sharding: data-parallel over batch -- core b processes sample b (no comms).

Per-core dataflow (sample = 32 channels x 1MB planes):
  - load channel plane i (1MB, contiguous) into SBUF tile [128, 2048]f32
    (partition p holds input rows 4p..4p+3)
  - 4 strided DVE copies (one per (dh,dw)) deinterleave into a staging tile
    laid out [p][c:4][hh:2][w:256] (partition p holds output rows 2p, 2p+1
    of all 4 output channels 4i..4i+3)
  - store staging tile as one 1MB DMA to out[4i:4i+4] (2KB contiguous runs)
"""

import numpy as np

from concourse import bacc, mybir, tile
from concourse.bass_utils import run_bass_kernel_spmd

B, C, H, W = 8, 32, 512, 512
N_CORES = 8

_cache = {}


def _build_nc(finalize=True, reps=1):
    nc = bacc.Bacc(
        "TRN2", target_bir_lowering=False, debug=False, num_devices=N_CORES
    )
    x = nc.dram_tensor("x", [C, H, W], mybir.dt.float32, kind="ExternalInput")
    out = nc.dram_tensor(
        "out", [4 * C, H // 2, W // 2], mybir.dt.float32, kind="ExternalOutput"
    )
    xa, oa = x.ap(), out.ap()

    with tile.TileContext(nc) as tc:
        with (
            tc.tile_pool(name="inp", bufs=3) as ip,
            tc.tile_pool(name="stg", bufs=3) as sp,
        ):
            for _ in range(reps):
                for i in range(C):
                    t = ip.tile([128, 2048], mybir.dt.float32)
                    # partition p <- x[i, 4p:4p+4, :] (8KB contiguous per partition)
                    nc.sync.dma_start(
                        t[:], xa[i].rearrange("(p r) w -> p (r w)", p=128)
                    )
                    s = sp.tile([128, 2048], mybir.dt.float32)
                    t3 = t[:].rearrange("p (j w) -> p j w", j=4)
                    s4 = s[:].rearrange("p (c hh w) -> p c hh w", c=4, hh=2)
                    for dh in range(2):
                        for dw in range(2):
                            nc.vector.tensor_copy(
                                s4[:, 2 * dh + dw], t3[:, dh::2, dw::2]
                            )
                    # staging partition p rows (2p, 2p+1) -> 2KB contiguous runs
                    nc.sync.dma_start(
                        oa[4 * i : 4 * i + 4].rearrange(
                            "c (p hh) w -> p c (hh w)", p=128, hh=2
                        ),
                        s[:].rearrange("p (c q) -> p c q", c=4),
                    )
    if finalize:
        nc.finalize()
    return nc


def kernel(x: np.ndarray) -> np.ndarray:
    assert x.shape == (B, C, H, W), x.shape
    if "nc" not in _cache:
        _cache["nc"] = _build_nc()
    nc = _cache["nc"]
    x = np.ascontiguousarray(np.asarray(x, dtype=np.float32))
    in_maps = [{"x": x[b]} for b in range(N_CORES)]
    res = run_bass_kernel_spmd(nc, in_maps, core_ids=list(range(N_CORES)))
    return np.stack([res.results[b]["out"] for b in range(N_CORES)], axis=0)


# revision 5
# speedup vs baseline: 1.1379x; 1.1379x over previous
"""Trainium2 Bass kernel for space-to-depth (pixel-unshuffle, factor 2).

Input  x:   (8, 32, 512, 512) f32
Output out: (8, 128, 256, 256) f32 with out[b, 4i+2dh+dw, h, w] = x[b, i, 2h+dh, 2w+dw]

Sharding: data-parallel over batch -- core b processes sample b (no comms).

Per-core dataflow (sample = 32 channels x 1MB planes):
  - load channel plane i (1MB, contiguous) into SBUF tile [128, 2048]f32
    (partition p holds input rows 4p..4p+3)
  - 4 strided DVE copies (one per (dh,dw)) deinterleave into a staging tile
    laid out [p][c:4][hh:2][w:256] (partition p holds output rows 2p, 2p+1
    of all 4 output channels 4i..4i+3)
  - store staging tile as one 1MB DMA to out[4i:4i+4] (2KB contiguous runs)
"""

import numpy as np

from concourse import bacc, mybir, tile
from concourse.bass_utils import run_bass_kernel_spmd

B, C, H, W = 8, 32, 512, 512
N_CORES = 8

_cache = {}


def _build_nc(finalize=True, reps=1, variant="v2", bufs=2):
    nc = bacc.Bacc(
        "TRN2", target_bir_lowering=False, debug=False, num_devices=N_CORES
    )
    x = nc.dram_tensor("x", [C, H, W], mybir.dt.float32, kind="ExternalInput")
    out = nc.dram_tensor(
        "out", [4 * C, H // 2, W // 2], mybir.dt.float32, kind="ExternalOutput"
    )
    xa, oa = x.ap(), out.ap()

    with tile.TileContext(nc) as tc:
        if variant == "v1":
            _emit_v1(nc, tc, xa, oa, reps)
        else:
            _emit_v2(nc, tc, xa, oa, reps, bufs)
    if finalize:
        nc.finalize()
    return nc


def _emit_v1(nc, tc, xa, oa, reps):
    """1 channel per tile: 1MB loads (8KB descs), 1MB stores (2KB descs)."""
    with (
        tc.tile_pool(name="inp", bufs=3) as ip,
        tc.tile_pool(name="stg", bufs=3) as sp,
    ):
        for _ in range(reps):
            for i in range(C):
                t = ip.tile([128, 2048], mybir.dt.float32)
                # partition p <- x[i, 4p:4p+4, :] (8KB contiguous per partition)
                nc.sync.dma_start(
                    t[:], xa[i].rearrange("(p r) w -> p (r w)", p=128)
                )
                s = sp.tile([128, 2048], mybir.dt.float32)
                t3 = t[:].rearrange("p (j w) -> p j w", j=4)
                s4 = s[:].rearrange("p (c hh w) -> p c hh w", c=4, hh=2)
                for dh in range(2):
                    for dw in range(2):
                        nc.vector.tensor_copy(
                            s4[:, 2 * dh + dw], t3[:, dh::2, dw::2]
                        )
                # staging partition p rows (2p, 2p+1) -> 2KB contiguous runs
                nc.sync.dma_start(
                    oa[4 * i : 4 * i + 4].rearrange(
                        "c (p hh) w -> p c (hh w)", p=128, hh=2
                    ),
                    s[:].rearrange("p (c q) -> p c q", c=4),
                )


def _emit_v2(nc, tc, xa, oa, reps, bufs):
    """4 channels per tile (4MB): 8KB descriptors on BOTH load and store;
    loads on the SP HWDGE ring, stores on the ACT ring.

    Tile partition p = (ci=p>>5, pp=p&31) holds x[4g+ci, 16pp:16pp+16, :]
    (32KB contiguous).  Staging partition p holds, for each co in 0..3,
    out[4*(4g+ci)+co, 8pp:8pp+8, :] as one 8KB contiguous run.
    """
    G = C // 4  # 8 groups
    with (
        tc.tile_pool(name="inp", bufs=bufs) as ip,
        tc.tile_pool(name="stg", bufs=bufs) as sp,
    ):
        for _ in range(reps):
            for g in range(G):
                t = ip.tile([128, 8192], mybir.dt.float32)
                nc.sync.dma_start(
                    t[:],
                    xa[4 * g : 4 * g + 4].rearrange(
                        "ci (pp r) w -> (ci pp) (r w)", pp=32
                    ),
                )
                s = sp.tile([128, 8192], mybir.dt.float32)
                t3 = t[:].rearrange("p (j w) -> p j w", j=16)
                s4 = s[:].rearrange("p (co hh w) -> p co hh w", co=4, hh=8)
                for dh in range(2):
                    for dw in range(2):
                        nc.vector.tensor_copy(
                            s4[:, 2 * dh + dw], t3[:, dh::2, dw::2]
                        )
                for ci in range(4):
                    c0 = 16 * g + 4 * ci
                    nc.scalar.dma_start(
                        oa[c0 : c0 + 4].rearrange(
                            "co (pp hh) w -> pp co (hh w)", hh=8
                        ),
                        s[32 * ci : 32 * ci + 32].rearrange(
                            "p (co q) -> p co q", co=4
                        ),
                    )


def kernel(x: np.ndarray) -> np.ndarray:
    assert x.shape == (B, C, H, W), x.shape
    if "nc" not in _cache:
        _cache["nc"] = _build_nc()
    nc = _cache["nc"]
    x = np.ascontiguousarray(np.asarray(x, dtype=np.float32))
    in_maps = [{"x": x[b]} for b in range(N_CORES)]
    res = run_bass_kernel_spmd(nc, in_maps, core_ids=list(range(N_CORES)))
    return np.stack([res.results[b]["out"] for b in range(N_CORES)], axis=0)


# revision 6
# speedup vs baseline: 1.2041x; 1.0581x over previous
"""Trainium2 Bass kernel for space-to-depth (pixel-unshuffle, factor 2).

Input  x:   (8, 32, 512, 512) f32
Output out: (8, 128, 256, 256) f32 with out[b, 4i+2dh+dw, h, w] = x[b, i, 2h+dh, 2w+dw]

Sharding: data-parallel over batch -- core b processes sample b (no comms).

Per-core dataflow (sample = 32 channels x 1MB planes):
  - load channel plane i (1MB, contiguous) into SBUF tile [128, 2048]f32
    (partition p holds input rows 4p..4p+3)
  - 4 strided DVE copies (one per (dh,dw)) deinterleave into a staging tile
    laid out [p][c:4][hh:2][w:256] (partition p holds output rows 2p, 2p+1
    of all 4 output channels 4i..4i+3)
  - store staging tile as one 1MB DMA to out[4i:4i+4] (2KB contiguous runs)
"""

import numpy as np

from concourse import bacc, mybir, tile
from concourse.bass_utils import run_bass_kernel_spmd

B, C, H, W = 8, 32, 512, 512
N_CORES = 8

_cache = {}


def _build_nc(finalize=True, reps=1, variant="v2", bufs=2):
    nc = bacc.Bacc(
        "TRN2", target_bir_lowering=False, debug=False, num_devices=N_CORES
    )
    x = nc.dram_tensor("x", [C, H, W], mybir.dt.float32, kind="ExternalInput")
    out = nc.dram_tensor(
        "out", [4 * C, H // 2, W // 2], mybir.dt.float32, kind="ExternalOutput"
    )
    xa, oa = x.ap(), out.ap()

    with tile.TileContext(nc) as tc:
        if variant == "v1":
            _emit_v1(nc, tc, xa, oa, reps)
        else:
            _emit_v2(nc, tc, xa, oa, reps, bufs)
    if finalize:
        nc.finalize()
    return nc


def _emit_v1(nc, tc, xa, oa, reps):
    """1 channel per tile: 1MB loads (8KB descs), 1MB stores (2KB descs)."""
    with (
        tc.tile_pool(name="inp", bufs=3) as ip,
        tc.tile_pool(name="stg", bufs=3) as sp,
    ):
        for _ in range(reps):
            for i in range(C):
                t = ip.tile([128, 2048], mybir.dt.float32)
                # partition p <- x[i, 4p:4p+4, :] (8KB contiguous per partition)
                nc.sync.dma_start(
                    t[:], xa[i].rearrange("(p r) w -> p (r w)", p=128)
                )
                s = sp.tile([128, 2048], mybir.dt.float32)
                t3 = t[:].rearrange("p (j w) -> p j w", j=4)
                s4 = s[:].rearrange("p (c hh w) -> p c hh w", c=4, hh=2)
                for dh in range(2):
                    for dw in range(2):
                        nc.vector.tensor_copy(
                            s4[:, 2 * dh + dw], t3[:, dh::2, dw::2]
                        )
                # staging partition p rows (2p, 2p+1) -> 2KB contiguous runs
                nc.sync.dma_start(
                    oa[4 * i : 4 * i + 4].rearrange(
                        "c (p hh) w -> p c (hh w)", p=128, hh=2
                    ),
                    s[:].rearrange("p (c q) -> p c q", c=4),
                )


def _emit_v2(nc, tc, xa, oa, reps, bufs):
    """4 channels per tile (4MB): 8KB descriptors on BOTH load and store;
    loads on the SP HWDGE ring, stores on the ACT ring.

    Tile partition p = (ci=p>>5, pp=p&31) holds x[4g+ci, 16pp:16pp+16, :]
    (32KB contiguous).  Staging partition p holds, for each co in 0..3,
    out[4*(4g+ci)+co, 8pp:8pp+8, :] as one 8KB contiguous run.
    """
    G = C // 4  # 8 groups
    if isinstance(bufs, int):
        bufs = (bufs, bufs)
    with (
        tc.tile_pool(name="inp", bufs=bufs[0]) as ip,
        tc.tile_pool(name="stg", bufs=bufs[1]) as sp,
    ):
        for _ in range(reps):
            for g in range(G):
                t = ip.tile([128, 8192], mybir.dt.float32)
                nc.sync.dma_start(
                    t[:],
                    xa[4 * g : 4 * g + 4].rearrange(
                        "ci (pp r) w -> (ci pp) (r w)", pp=32
                    ),
                )
                s = sp.tile([128, 8192], mybir.dt.float32)
                t3 = t[:].rearrange("p (j w) -> p j w", j=16)
                s4 = s[:].rearrange("p (co hh w) -> p co hh w", co=4, hh=8)
                for dh in range(2):
                    for dw in range(2):
                        nc.vector.tensor_copy(
                            s4[:, 2 * dh + dw], t3[:, dh::2, dw::2]
                        )
                for ci in range(4):
                    c0 = 16 * g + 4 * ci
                    nc.scalar.dma_start(
                        oa[c0 : c0 + 4].rearrange(
                            "co (pp hh) w -> pp co (hh w)", hh=8
                        ),
                        s[32 * ci : 32 * ci + 32].rearrange(
                            "p (co q) -> p co q", co=4
                        ),
                    )


def kernel(x: np.ndarray) -> np.ndarray:
    assert x.shape == (B, C, H, W), x.shape
    if "nc" not in _cache:
        _cache["nc"] = _build_nc()
    nc = _cache["nc"]
    x = np.ascontiguousarray(np.asarray(x, dtype=np.float32))
    in_maps = [{"x": x[b]} for b in range(N_CORES)]
    res = run_bass_kernel_spmd(nc, in_maps, core_ids=list(range(N_CORES)))
    return np.stack([res.results[b]["out"] for b in range(N_CORES)], axis=0)
